# revision 1
# baseline (speedup 1.0000x reference)
"""Trainium2 Bass kernel for nn_Attention3D (GroupNorm + channel-attention + proj + residual).

Sharding: the spatial axis N = d*h*w = 32768 is split across 8 cores (Nc=4096
per core, both batch elements on every core). Two tiny AllReduces:
  AR1: per-channel GroupNorm partial stats (mean, E[x^2])      [128 x 8]  f32
  AR2: channel-attention logits q @ k^T (contracted over N)    [128 x 256] f32

Key algebraic fusions (validated against the reference in numpy):
  - GroupNorm affine is folded into the q/k weight matrix (per-batch row
    scaling) so normalized activations are never materialized.
  - softmax(attn) @ v followed by proj collapses into a single per-batch
    weight G_b = P @ blockdiag(attn) @ Wv (256x256), applied directly to raw
    x, with a per-batch bias vector carrying all bias/affine terms.
  - qkv bias + GroupNorm shift enter the logits as rank-1 corrections added
    after AR2 (exact, from globally-reduced column sums).
"""
import sys

sys.path.insert(0, "/opt/trn_rl_repo")

import numpy as np
import concourse.bass as bass
import concourse.tile as tile
from concourse import mybir
from concourse.bass_utils import run_bass_kernel_spmd

F32 = mybir.dt.float32
F32R = mybir.dt.float32r
ALU = mybir.AluOpType
ACT = mybir.ActivationFunctionType

S = 8            # cores
B, C = 2, 256
N = 32 * 32 * 32
Nc = N // S      # 4096 spatial positions per core
H, HD = 4, 64
G = 8            # groupnorm groups
EPS = 1e-5
SM_SCALE = float(HD) ** -0.5


def _split_excess_waits(nc, max_waits=1):
    """This container's walrus rejects >1 sem wait per instruction; move the
    overflow onto same-engine NoOps inserted immediately before."""
    ctr = 0
    for bb in nc.cur_f.blocks:
        insts = bb.instructions
        i = 0
        while i < len(insts):
            ins = insts[i]
            si = ins.sync_info
            if si is not None and len(si.on_wait) > max_waits:
                waits = list(si.on_wait)
                si.on_wait = waits[:max_waits]
                overflow = waits[max_waits:]
                pos = i
                for j in range(0, len(overflow), max_waits):
                    ctr += 1
                    nop = mybir.InstNoOp(name=f"I-ws-{ctr}", ins=[], outs=[])
                    nop.engine = ins.engine
                    nop.sync_info = mybir.SyncInfo(
                        on_wait=overflow[j : j + max_waits], on_update=[]
                    )
                    insts.insert(pos, nop)
                    pos += 1
                    i += 1
            i += 1


def build_nc(split_waits=True, loop_r=None, upto=99):
    """loop_r=None builds the real kernel. loop_r=R builds a timing variant:
    collectives run once up-front, then the full compute body repeats R times
    inside a hardware For_i loop (for wall-clock slope measurements).
    upto (timing variant only): emit only loop-body phases <= upto:
      0=x reload, 1=stats, 2=post-AR1 prep, 3=pass1, 4=extract+ccdma,
      5=softmax, 6=fused weights, 7=pass2+out."""
    nc = bass.Bass(num_devices=S)

    xs_d = nc.declare_dram_parameter("xs", [2 * B, 128, Nc], F32R, isOutput=False)
    wtqk_d = nc.declare_dram_parameter("wtqk", [C, 512], F32R, isOutput=False)
    wv_d = nc.declare_dram_parameter("wv", [C, C], F32R, isOutput=False)
    pt_d = nc.declare_dram_parameter("pt", [C, C], F32R, isOutput=False)
    gnw_d = nc.declare_dram_parameter("gnw", [C, 1], F32, isOutput=False)
    gnb_d = nc.declare_dram_parameter("gnb", [C, 1], F32, isOutput=False)
    bqk_d = nc.declare_dram_parameter("bqk", [1, 512], F32R, isOutput=False)
    bv_d = nc.declare_dram_parameter("bv", [C, 1], F32R, isOutput=False)
    pb_d = nc.declare_dram_parameter("pb", [1, C], F32, isOutput=False)
    g4_d = nc.declare_dram_parameter("g4", [128, 4], F32, isOutput=False)
    e4_d = nc.declare_dram_parameter("e4", [4, 128], F32, isOutput=False)
    const_d = nc.declare_dram_parameter("konst", [128, 257], F32R, isOutput=False)
    out_d = nc.declare_dram_parameter("out", [2 * B, 128, Nc], F32, isOutput=True)

    cc1i = nc.dram_tensor("cc1i", [128, 8], F32)
    cc1o = nc.dram_tensor("cc1o", [128, 8], F32, addr_space="Shared")
    cc2i = nc.dram_tensor("cc2i", [128, 256], F32)
    cc2o = nc.dram_tensor("cc2o", [128, 256], F32, addr_space="Shared")
    rg = [list(range(S))]

    with tile.TileContext(nc) as tc:
        with (
            tc.tile_pool(name="big", bufs=1) as big,        # resident x / out
            tc.tile_pool(name="wpool", bufs=1) as wpool,    # weights & per-batch mats
            tc.tile_pool(name="small", bufs=1) as small,    # stats / vectors
            tc.tile_pool(name="qkpool", bufs=3) as qkpool,  # pass-1 qk^T staging
            tc.tile_pool(name="p_att", bufs=1, space="PSUM") as p_att,
            tc.tile_pool(name="p_work", bufs=2, space="PSUM") as p_work,
            tc.tile_pool(name="p_misc", bufs=2, space="PSUM") as p_misc,
        ):
            # ---------- phase 0: loads ----------
            x_sb = []  # t = b*2+cb -> [128, Nc]
            for t in range(4):
                xt = big.tile([128, Nc], F32R, tag=f"x{t}", name=f"x{t}")
                nc.sync.dma_start(out=xt[:], in_=xs_d[t])
                x_sb.append(xt)
            wtqk_sb = []
            for k in range(2):
                w = wpool.tile([128, 512], F32R, tag=f"wtqk{k}", name=f"wtqk{k}")
                nc.sync.dma_start(out=w[:], in_=wtqk_d[k * 128:(k + 1) * 128, :])
                wtqk_sb.append(w)
            wv_sb, pt_sb = [], []
            for k in range(2):
                w = wpool.tile([128, C], F32R, tag=f"wv{k}", name=f"wv{k}")
                nc.sync.dma_start(out=w[:], in_=wv_d[k * 128:(k + 1) * 128, :])
                wv_sb.append(w)
                p = wpool.tile([128, C], F32R, tag=f"pt{k}", name=f"pt{k}")
                nc.sync.dma_start(out=p[:], in_=pt_d[k * 128:(k + 1) * 128, :])
                pt_sb.append(p)
            gnw_sb, gnb_sb, bv_sb = [], [], []
            for k in range(2):
                sl = slice(k * 128, (k + 1) * 128)
                gw = small.tile([128, 1], F32, tag=f"gnw{k}", name=f"gnw{k}")
                nc.sync.dma_start(out=gw[:], in_=gnw_d[sl, :])
                gnw_sb.append(gw)
                gb = small.tile([128, 1], F32, tag=f"gnb{k}", name=f"gnb{k}")
                nc.sync.dma_start(out=gb[:], in_=gnb_d[sl, :])
                gnb_sb.append(gb)
                bv = small.tile([128, 1], F32R, tag=f"bv{k}", name=f"bv{k}")
                nc.sync.dma_start(out=bv[:], in_=bv_d[sl, :])
                bv_sb.append(bv)

            pb_sb = small.tile([1, C], F32, tag="pb", name="pb")
            nc.sync.dma_start(out=pb_sb[:], in_=pb_d[:])
            bqk_sb = small.tile([1, 512], F32R, tag="bqk", name="bqk")
            nc.sync.dma_start(out=bqk_sb[:], in_=bqk_d[:])
            g4_sb = small.tile([128, 4], F32, tag="g4", name="g4")
            nc.sync.dma_start(out=g4_sb[:], in_=g4_d[:])
            e4_sb = small.tile([4, 128], F32, tag="e4", name="e4")
            nc.sync.dma_start(out=e4_sb[:], in_=e4_d[:])

            eps41 = small.tile([4, 1], F32, tag="eps", name="eps")
            nc.gpsimd.memset(eps41[:], EPS)
            konst_sb = wpool.tile([128, 257], F32R, tag="konst", name="konst")
            nc.sync.dma_start(out=konst_sb[:], in_=const_d[:])
            one11 = konst_sb[0:1, 256:257]
            scr41 = small.tile([4, 1], F32, tag="scr", name="scr")
            # preload the sqrt activation table while DMAs run
            nc.scalar.activation(out=scr41[:], in_=eps41[:], func=ACT.Sqrt)

            def emit_stats():
                """phase 1: local GroupNorm stats -> st [128, 8] -> cc1i."""
                st = small.tile([128, 8], F32, tag="st", name="st")
                for t in range(4):
                    stats6 = small.tile([128, 8, 6], F32, tag="bn6", name="bn6")
                    for j in range(8):
                        nc.vector.bn_stats(
                            out=stats6[:, j, :], in_=x_sb[t][:, j * 512:(j + 1) * 512]
                        )
                    mv = small.tile([128, 2], F32, tag="mv", name="mv")
                    nc.vector.bn_aggr(out=mv[:], in_=stats6[:])
                    nc.vector.tensor_copy(st[:, t:t + 1], mv[:, 0:1])
                    # E[x^2] = var + mean^2
                    nc.vector.scalar_tensor_tensor(
                        out=st[:, 4 + t:5 + t], in0=mv[:, 0:1], scalar=mv[:, 0:1],
                        in1=mv[:, 1:2], op0=ALU.mult, op1=ALU.add,
                    )
                nc.sync.dma_start(out=cc1i[:], in_=st[:])

            def emit_compute(upto=99):
                """phases 2..7 (generator; yields where AR2 belongs)."""
                st2 = small.tile([128, 8], F32, tag="st2", name="st2")
                nc.sync.dma_start(out=st2[:], in_=cc1o[:])

                # ----- post-AR1 prep -----
                psum_g = p_misc.tile([4, 8], F32, tag="m", name="psum_g")
                nc.tensor.matmul(psum_g[:], g4_sb[:], st2[:], start=True, stop=True)
                gsb = small.tile([4, 8], F32, tag="gsb", name="gsb")
                nc.vector.tensor_copy(gsb[:], psum_g[:])
                var44 = small.tile([4, 4], F32, tag="var44", name="var44")
                nc.vector.scalar_tensor_tensor(
                    out=var44[:], in0=gsb[:, 0:4], scalar=0.0, in1=gsb[:, 0:4],
                    op0=ALU.add, op1=ALU.mult,
                )  # mean^2
                nc.vector.tensor_sub(var44[:], gsb[:, 4:8], var44[:])
                rstd44 = small.tile([4, 4], F32, tag="rstd44", name="rstd44")
                nc.scalar.activation(
                    out=rstd44[:], in_=var44[:], func=ACT.Sqrt, bias=eps41[:], scale=1.0
                )
                nc.vector.reciprocal(out=rstd44[:], in_=rstd44[:])
                # preload the exp table right after the last sqrt
                nc.scalar.activation(out=scr41[:], in_=rstd44[:, 0:1], func=ACT.Exp)

                a_sb = [[None] * 2 for _ in range(B)]
                bb_sb = [[None] * 2 for _ in range(B)]
                wts_sb = [[None] * 2 for _ in range(B)]
                sxg_sb = [[None] * 2 for _ in range(B)]
                for b in range(B):
                    for cb in range(2):
                        t = b * 2 + cb
                        pmean = p_misc.tile([128, 1], F32, tag="m", name="pmean")
                        nc.tensor.matmul(
                            pmean[:], e4_sb[:], gsb[:, t:t + 1], start=True, stop=True
                        )
                        prstd = p_misc.tile([128, 1], F32, tag="m", name="prstd")
                        nc.tensor.matmul(
                            prstd[:], e4_sb[:], rstd44[:, t:t + 1], start=True, stop=True
                        )
                        a = small.tile([128, 1], F32, tag=f"a{t}", name=f"a{t}")
                        nc.vector.tensor_mul(a[:], prstd[:], gnw_sb[cb][:])
                        na = small.tile([128, 1], F32, tag=f"na{t}", name=f"na{t}")
                        nc.scalar.mul(out=na[:], in_=a[:], mul=-1.0)
                        bbv = small.tile([128, 1], F32R, tag=f"bb{t}", name=f"bb{t}")
                        nc.vector.scalar_tensor_tensor(
                            out=bbv[:], in0=pmean[:], scalar=na[:], in1=gnb_sb[cb][:],
                            op0=ALU.mult, op1=ALU.add,
                        )  # gnb - mean*a
                        w = wpool.tile([128, 512], F32R, tag=f"wts{t}", name=f"wts{t}")
                        nc.vector.tensor_scalar_mul(out=w[:], in0=wtqk_sb[cb][:], scalar1=a[:])
                        sx = small.tile([128, 1], F32R, tag=f"sxg{t}", name=f"sxg{t}")
                        nc.scalar.mul(out=sx[:], in_=st2[:, t:t + 1], mul=float(Nc))
                        a_sb[b][cb], bb_sb[b][cb], wts_sb[b][cb], sxg_sb[b][cb] = a, bbv, w, sx

                if upto < 3:
                    return
                # rowbias rb, global colsums Sg, and the rank-1 stacks Lq/Rk
                lq_sb, rk_sb = [], []
                for b in range(B):
                    prb = p_misc.tile([1, 512], F32, tag="m", name="prb")
                    nc.tensor.matmul(prb[:], bb_sb[b][0][:], wtqk_sb[0][:], start=True, stop=False)
                    nc.tensor.matmul(prb[:], bb_sb[b][1][:], wtqk_sb[1][:], start=False, stop=False)
                    nc.tensor.matmul(prb[:], one11, bqk_sb[:], start=False, stop=True)
                    rb = small.tile([1, 512], F32, tag=f"rb{b}", name=f"rb{b}")
                    nc.vector.tensor_copy(rb[:], prb[:])
                    psg = p_misc.tile([1, 512], F32, tag="m", name="psg")
                    nc.tensor.matmul(psg[:], sxg_sb[b][0][:], wts_sb[b][0][:], start=True, stop=False)
                    nc.tensor.matmul(psg[:], sxg_sb[b][1][:], wts_sb[b][1][:], start=False, stop=True)
                    sg = small.tile([1, 512], F32, tag=f"sg{b}", name=f"sg{b}")
                    nc.vector.tensor_copy(sg[:], psg[:])
                    rbn = small.tile([1, 512], F32, tag=f"rbn{b}", name=f"rbn{b}")
                    nc.scalar.mul(out=rbn[:], in_=rb[:], mul=float(N))
                    lq = small.tile([3, 256], F32, tag=f"lq{b}", name=f"lq{b}")
                    nc.sync.dma_start(out=lq[0:1, :], in_=rb[0:1, 0:256])
                    nc.sync.dma_start(out=lq[1:2, :], in_=sg[0:1, 0:256])
                    nc.sync.dma_start(out=lq[2:3, :], in_=rbn[0:1, 0:256])
                    rk = small.tile([3, 256], F32, tag=f"rk{b}", name=f"rk{b}")
                    nc.sync.dma_start(out=rk[0:1, :], in_=sg[0:1, 256:512])
                    nc.sync.dma_start(out=rk[1:2, :], in_=rb[0:1, 256:512])
                    nc.sync.dma_start(out=rk[2:3, :], in_=rb[0:1, 256:512])
                    lq_sb.append(lq)
                    rk_sb.append(rk)

                # ----- pass 1: q/k logits -----
                att_ps = [
                    [
                        p_att.tile([128, 256], F32, tag=f"att{b}{hp}", name=f"att{b}{hp}")
                        for hp in range(2)
                    ]
                    for b in range(B)
                ]
                for b in range(B):
                    for i in range(Nc // 128):
                        nsl = slice(i * 128, (i + 1) * 128)
                        pqk = p_work.tile([128, 512], F32, tag="w", name="pqk")
                        nc.tensor.matmul(
                            pqk[:], x_sb[b * 2][:, nsl], wts_sb[b][0][:], start=True, stop=False
                        )
                        nc.tensor.matmul(
                            pqk[:], x_sb[b * 2 + 1][:, nsl], wts_sb[b][1][:], start=False, stop=True
                        )
                        qkt = qkpool.tile([128, 512], F32R, tag="qkt", name="qkt")
                        if i % 2 == 0:
                            nc.vector.tensor_copy(qkt[:], pqk[:])
                        else:
                            nc.scalar.copy(out=qkt[:], in_=pqk[:])
                        first, last = i == 0, i == Nc // 128 - 1
                        for hp in range(2):
                            nc.tensor.matmul(
                                att_ps[b][hp][:],
                                qkt[:, hp * 128:(hp + 1) * 128],
                                qkt[:, 256:512],
                                start=first, stop=last,
                            )

                if upto < 4:
                    return
                # ----- extract diag blocks -> cc2i -----
                att_all = small.tile([128, 256], F32, tag="att_all", name="att_all")
                for b in range(B):
                    for hp in range(2):
                        t2 = 2 * b + hp
                        csl = slice(t2 * 64, (t2 + 1) * 64)
                        so = hp * 128
                        nc.vector.tensor_copy(att_all[0:64, csl], att_ps[b][hp][0:64, so:so + 64])
                        nc.vector.tensor_copy(att_all[64:128, csl], att_ps[b][hp][64:128, so + 64:so + 128])
                nc.sync.dma_start(out=cc2i[:], in_=att_all[:])
                yield  # AllReduce of cc2i -> cc2o happens here (real kernel)
                attg = small.tile([128, 256], F32, tag="attg", name="attg")
                nc.sync.dma_start(out=attg[:], in_=cc2o[:])

                if upto < 5:
                    return
                # ----- bias corrections + softmax -----
                att_sm = [[None] * 2 for _ in range(B)]
                for b in range(B):
                    for hp in range(2):
                        t2 = 2 * b + hp
                        pc = p_misc.tile([128, 64], F32, tag="m", name="pc")
                        for hh in range(2):
                            h = 2 * hp + hh
                            hsl = slice(h * 64, (h + 1) * 64)
                            nc.tensor.matmul(
                                pc[hh * 64:(hh + 1) * 64, :],
                                lq_sb[b][:, hsl], rk_sb[b][:, hsl],
                                start=True, stop=True, skip_group_check=True,
                            )
                        atc = small.tile([128, 64], F32, tag="atc", name="atc")
                        nc.vector.tensor_add(atc[:], attg[:, t2 * 64:(t2 + 1) * 64], pc[:])
                        negm = small.tile([128, 1], F32, tag="negm", name="negm")
                        nc.vector.reduce_max(
                            out=negm[:], in_=atc[:], axis=mybir.AxisListType.X, negate=True
                        )
                        nc.scalar.mul(out=negm[:], in_=negm[:], mul=SM_SCALE)
                        esb = small.tile([128, 64], F32, tag="esb", name="esb")
                        nc.scalar.activation(
                            out=esb[:], in_=atc[:], func=ACT.Exp,
                            bias=negm[:], scale=SM_SCALE,
                        )
                        ssum = small.tile([128, 1], F32, tag="ssum", name="ssum")
                        nc.vector.reduce_sum(out=ssum[:], in_=esb[:], axis=mybir.AxisListType.X)
                        nc.vector.reciprocal(out=ssum[:], in_=ssum[:])
                        sm = small.tile([128, 64], F32, tag=f"sm{t2}", name=f"sm{t2}")
                        nc.vector.tensor_scalar_mul(out=sm[:], in0=esb[:], scalar1=ssum[:])
                        att_sm[b][hp] = sm

                if upto < 6:
                    return
                # ----- blockdiag + fused per-batch weights -----
                gbt_sb = [[None] * 2 for _ in range(B)]
                mbt_sb = [[None] * 2 for _ in range(B)]
                beta_sb = [[None] * 2 for _ in range(B)]
                for b in range(B):
                    ablk = []
                    for k in range(2):
                        ab = wpool.tile([128, 256], F32R, tag=f"ablk{b}{k}", name=f"ablk{b}{k}")
                        nc.vector.tensor_copy(ab[:], konst_sb[:, 0:256])
                        h0, h1 = 2 * k, 2 * k + 1
                        nc.vector.tensor_copy(ab[0:64, h0 * 64:(h0 + 1) * 64], att_sm[b][k][0:64, :])
                        nc.vector.tensor_copy(ab[64:128, h1 * 64:(h1 + 1) * 64], att_sm[b][k][64:128, :])
                        ablk.append(ab)
                    for m in range(2):
                        pm = p_misc.tile([128, 256], F32, tag="m", name="pm")
                        msl = slice(m * 128, (m + 1) * 128)
                        nc.tensor.matmul(pm[:], ablk[0][:, msl], pt_sb[0][:], start=True, stop=False)
                        nc.tensor.matmul(pm[:], ablk[1][:, msl], pt_sb[1][:], start=False, stop=True)
                        mbt = wpool.tile([128, 256], F32R, tag=f"mbt{b}{m}", name=f"mbt{b}{m}")
                        nc.vector.tensor_copy(mbt[:], pm[:])
                        mbt_sb[b][m] = mbt
                    for g in range(2):
                        pg2 = p_misc.tile([128, 256], F32, tag="m", name="pg2")
                        gsl = slice(g * 128, (g + 1) * 128)
                        nc.tensor.matmul(pg2[:], wv_sb[0][:, gsl], mbt_sb[b][0][:], start=True, stop=False)
                        nc.tensor.matmul(pg2[:], wv_sb[1][:, gsl], mbt_sb[b][1][:], start=False, stop=True)
                        gbt = wpool.tile([128, 256], F32R, tag=f"gbt{b}{g}", name=f"gbt{b}{g}")
                        nc.vector.tensor_copy(gbt[:], pg2[:])
                        gbt_sb[b][g] = gbt
                    pbeta = p_misc.tile([1, C], F32, tag="m", name="pbeta")
                    nc.tensor.matmul(pbeta[:], bb_sb[b][0][:], gbt_sb[b][0][:], start=True, stop=False)
                    nc.tensor.matmul(pbeta[:], bb_sb[b][1][:], gbt_sb[b][1][:], start=False, stop=False)
                    nc.tensor.matmul(pbeta[:], bv_sb[0][:], mbt_sb[b][0][:], start=False, stop=False)
                    nc.tensor.matmul(pbeta[:], bv_sb[1][:], mbt_sb[b][1][:], start=False, stop=True)
                    brow = small.tile([1, C], F32, tag=f"brow{b}", name=f"brow{b}")
                    nc.vector.tensor_add(brow[:], pbeta[:], pb_sb[:])
                    for mo in range(2):
                        bet = small.tile([128, 1], F32, tag=f"beta{b}{mo}", name=f"beta{b}{mo}")
                        nc.sync.dma_start(out=bet[:], in_=brow[0:1, mo * 128:(mo + 1) * 128])
                        beta_sb[b][mo] = bet
                    # fold the GroupNorm scale into G_b (after the bias matmuls read it)
                    for g in range(2):
                        nc.vector.tensor_scalar_mul(
                            out=gbt_sb[b][g][:], in0=gbt_sb[b][g][:], scalar1=a_sb[b][g][:]
                        )

                if upto < 7:
                    return
                # ----- pass 2: out = G_b' x + beta + x -----
                for b in range(B):
                    for mo in range(2):
                        t = b * 2 + mo
                        osb = big.tile([128, Nc], F32, tag=f"o{t}", name=f"o{t}")
                        msl = slice(mo * 128, (mo + 1) * 128)
                        for nt in range(Nc // 512):
                            nsl = slice(nt * 512, (nt + 1) * 512)
                            po = p_work.tile([128, 512], F32, tag="w", name="po")
                            nc.tensor.matmul(po[:], gbt_sb[b][0][:, msl], x_sb[b * 2][:, nsl],
                                             start=True, stop=False)
                            nc.tensor.matmul(po[:], gbt_sb[b][1][:, msl], x_sb[b * 2 + 1][:, nsl],
                                             start=False, stop=True)
                            nc.vector.scalar_tensor_tensor(
                                out=osb[:, nsl], in0=po[:], scalar=beta_sb[b][mo][:],
                                in1=x_sb[t][:, nsl], op0=ALU.add, op1=ALU.add,
                            )
                        nc.sync.dma_start(out=out_d[t], in_=osb[:])

            def ar1():
                nc.gpsimd.collective_compute(
                    "AllReduce", ALU.add, replica_groups=rg, ins=[cc1i[:]], outs=[cc1o[:]]
                )

            def ar2():
                nc.gpsimd.collective_compute(
                    "AllReduce", ALU.add, replica_groups=rg, ins=[cc2i[:]], outs=[cc2o[:]]
                )

            if loop_r is None:
                emit_stats()
                ar1()
                gen = emit_compute()
                next(gen)          # everything up to (and incl.) the cc2i write
                ar2()
                for _ in gen:      # the rest
                    pass
            else:
                # timing variant: collectives once, compute body looped
                emit_stats()
                ar1()
                ar2()
                with tc.For_i(0, loop_r, 1):
                    for t in range(4):
                        nc.sync.dma_start(out=x_sb[t][:], in_=xs_d[t])
                    if upto >= 1:
                        emit_stats()
                    if upto >= 2:
                        for _ in emit_compute(upto):
                            pass

    if split_waits:
        _split_excess_waits(nc)
    return nc


_NC_CACHE = None


def _get_nc():
    global _NC_CACHE
    if _NC_CACHE is None:
        _NC_CACHE = build_nc()
    return _NC_CACHE


def _prep_inputs(x, gn_w, gn_b, qkv_w, qkv_b, proj_w, proj_b):
    x = np.ascontiguousarray(np.asarray(x, np.float32)).reshape(B, C, N)
    qkv_w = np.asarray(qkv_w, np.float32)
    qkv_b = np.asarray(qkv_b, np.float32)
    proj_w = np.asarray(proj_w, np.float32)
    shared = {
        "wtqk": np.ascontiguousarray(qkv_w[0:512].T),
        "wv": np.ascontiguousarray(qkv_w[512:768]),
        "pt": np.ascontiguousarray(proj_w.T),
        "gnw": np.asarray(gn_w, np.float32).reshape(C, 1),
        "gnb": np.asarray(gn_b, np.float32).reshape(C, 1),
        "bqk": qkv_b[0:512].reshape(1, 512),
        "bv": qkv_b[512:768].reshape(C, 1),
        "pb": np.asarray(proj_b, np.float32).reshape(1, C),
    }
    g4 = np.zeros((128, 4), np.float32)
    for p in range(128):
        g4[p, p // 32] = 1.0 / (32.0 * S)
    e4 = np.zeros((4, 128), np.float32)
    for p in range(128):
        e4[p // 32, p] = 1.0
    shared["g4"] = g4
    shared["e4"] = e4
    konst = np.zeros((128, 257), np.float32)
    konst[0, 256] = 1.0
    shared["konst"] = konst
    in_maps = []
    for s in range(S):
        xs = np.ascontiguousarray(x[:, :, s * Nc:(s + 1) * Nc]).reshape(2 * B, 128, Nc)
        in_maps.append({"xs": xs, **{k: v for k, v in shared.items()}})
    return in_maps


def kernel(x, gn_w, gn_b, qkv_w, qkv_b, proj_w, proj_b):
    nc = _get_nc()
    in_maps = _prep_inputs(x, gn_w, gn_b, qkv_w, qkv_b, proj_w, proj_b)
    res = run_bass_kernel_spmd(nc, in_maps, list(range(S)), trace=False)
    shards = [res.results[s]["out"].reshape(B, C, Nc) for s in range(S)]
    return np.concatenate(shards, axis=2).reshape(B, C, 32, 32, 32).astype(np.float32)



# revision 11
# speedup vs baseline: 1.1323x; 1.1323x over previous
"""Trainium2 Bass kernel for nn_Attention3D (GroupNorm + channel-attention + proj + residual).

Single-core design. Measurement showed the per-call wall time through the
axon/PJRT relay is dominated by a fixed ~68ms dispatch floor that grows with
the number of devices in the mesh (and with per-device buffer count), while
on-device compute for this problem is <1ms. So: run everything on ONE core
with zero collectives and only three input buffers.

Algorithm (validated against the reference in numpy, rel err ~2e-5):
  - Per batch, the channel Gram matrix G = x x^T [256,256] and row-sums
    s = x 1 are accumulated from a host-pretransposed copy of x (with a
    ones-column fused so one matmul pair yields both G and s).
  - GroupNorm stats come from s (means) and diag(G) (second moments), so the
    normalization is never materialized: its affine folds into the q/k
    weights (per-batch row scaling a) and rank-1 logit corrections.
  - Channel-attention logits = Wq_a G Wk_a^T + rank-1 corrections (from bias
    and GroupNorm shift terms), then per-head softmax on the [64,64] blocks.
  - softmax @ v + proj collapse into a per-batch weight G_b = P blockdiag(A)
    Wv (x scale fold), applied to raw x in one streamed pass with the
    residual and all bias terms fused.
"""
import sys

sys.path.insert(0, "/opt/trn_rl_repo")

import numpy as np
import concourse.bass as bass
import concourse.tile as tile
from concourse import mybir
from concourse.bass_utils import run_bass_kernel_spmd
from concourse.masks import make_identity

F32 = mybir.dt.float32
F32R = mybir.dt.float32r
ALU = mybir.AluOpType
ACT = mybir.ActivationFunctionType

B, C = 2, 256
N = 32 * 32 * 32
H, HD = 4, 64
G = 8
EPS = 1e-5
SM_SCALE = float(HD) ** -0.5
NT = N // 128          # 256 transposed n-tiles per batch
TB = 4                 # n-tiles per DMA in the Gram pass
CT = 1024              # column tile in the output pass

# konst column layout
WTQK0, WTQK1 = 0, 512            # [128,512] each: (qkv_w[0:512].T) chunks
WV0, WV1 = 1024, 1280            # [128,256] each: qkv_w[512:768] row chunks
PT0, PT1 = 1536, 1792            # [128,256] each: proj_w.T row chunks
ZEROS = 2048                     # [128,256] zeros
GNW, GNB, BVC = 2304, 2306, 2308  # [128,1] x2 each
G4 = 2310                        # [128,4] group reducer (1/(32N))
E4 = 2314                        # rows 0:4, [4,128] group broadcaster
BQK = 2442                       # rows 0:1, [1,512] qk bias
PB = 2954                        # rows 0:1, [1,256] proj bias
ONE = 3210                       # rows 0:1, [1,1] = 1.0
KC = 3212


def _split_excess_waits(nc, max_waits=1):
    """This container's walrus rejects >1 sem wait per instruction; move the
    overflow onto same-engine NoOps inserted immediately before."""
    ctr = 0
    for bb in nc.cur_f.blocks:
        insts = bb.instructions
        i = 0
        while i < len(insts):
            ins = insts[i]
            si = ins.sync_info
            if si is not None and len(si.on_wait) > max_waits:
                waits = list(si.on_wait)
                si.on_wait = waits[:max_waits]
                overflow = waits[max_waits:]
                pos = i
                for j in range(0, len(overflow), max_waits):
                    ctr += 1
                    nop = mybir.InstNoOp(name=f"I-ws-{ctr}", ins=[], outs=[])
                    nop.engine = ins.engine
                    nop.sync_info = mybir.SyncInfo(
                        on_wait=overflow[j : j + max_waits], on_update=[]
                    )
                    insts.insert(pos, nop)
                    pos += 1
                    i += 1
            i += 1


def build_nc():
    nc = bass.Bass(num_devices=1)

    xs_d = nc.declare_dram_parameter("xs", [2 * B, 128, N], F32R, isOutput=False)
    xt_d = nc.declare_dram_parameter("xt", [B, 128, NT * 258], F32R, isOutput=False)
    konst_d = nc.declare_dram_parameter("konst", [128, KC], F32R, isOutput=False)
    out_d = nc.declare_dram_parameter("out", [2 * B, 128, N], F32, isOutput=True)

    with tile.TileContext(nc) as tc:
        with (
            tc.tile_pool(name="wpool", bufs=1) as wpool,    # konst, wts, G, fused mats
            tc.tile_pool(name="small", bufs=1) as small,    # stats / vectors
            tc.tile_pool(name="xtp", bufs=3) as xtp,        # Gram-pass staging
            tc.tile_pool(name="xc0", bufs=2) as xc0,        # out-pass x chunk0
            tc.tile_pool(name="xc1", bufs=2) as xc1,        # out-pass x chunk1
            tc.tile_pool(name="op", bufs=2) as op,          # out-pass staging
            tc.tile_pool(name="p_big", bufs=2, space="PSUM") as p_big,
            tc.tile_pool(name="p_att", bufs=1, space="PSUM") as p_att,
            tc.tile_pool(name="p_misc", bufs=2, space="PSUM") as p_misc,
        ):
            # ---------- loads / constants ----------
            konst = wpool.tile([128, KC], F32R, tag="konst", name="konst")
            nc.sync.dma_start(out=konst[:], in_=konst_d[:])
            ident = small.tile([128, 128], F32, tag="ident", name="ident")
            make_identity(nc, ident[:])
            eps41 = small.tile([4, 1], F32, tag="eps", name="eps")
            nc.gpsimd.memset(eps41[:], EPS)
            scr41 = small.tile([4, 1], F32, tag="scr", name="scr")
            # preload activation tables while DMAs run
            nc.scalar.activation(out=scr41[:], in_=eps41[:], func=ACT.Sqrt)
            nc.scalar.activation(out=scr41[:], in_=eps41[:], func=ACT.Exp)

            # ---------- phase A: per-batch Gram G = x x^T and row-sums ----------
            gsb = [[None] * 2 for _ in range(B)]  # [b][half] -> [128,257] SBUF
            for b in range(B):
                g_ps = [
                    p_big.tile([128, 258], F32, tag="w", name=f"g{b}{m}")
                    for m in range(2)
                ]
                n_batches = NT // TB
                for jb in range(n_batches):
                    xt = xtp.tile([128, TB * 258], F32R, tag="xt", name=f"xt{b}{jb}")
                    nc.sync.dma_start(
                        out=xt[:], in_=xt_d[b][:, jb * TB * 258:(jb + 1) * TB * 258]
                    )
                    for k in range(TB):
                        first = jb == 0 and k == 0
                        last = jb == n_batches - 1 and k == TB - 1
                        base = k * 258
                        for m in range(2):
                            nc.tensor.matmul(
                                g_ps[m][:],
                                xt[:, base + m * 128: base + (m + 1) * 128],
                                xt[:, base: base + 258],
                                start=first, stop=last,
                            )
                for m in range(2):
                    gt = wpool.tile([128, 258], F32R, tag=f"G{b}{m}", name=f"G{b}{m}")
                    if m == 0:
                        nc.vector.tensor_copy(gt[:], g_ps[m][:])
                    else:
                        nc.scalar.copy(out=gt[:], in_=g_ps[m][:])
                    gsb[b][m] = gt

            # ---------- phase B1: GroupNorm stats from s and diag(G) ----------
            st = small.tile([128, 8], F32R, tag="st", name="st")
            for b in range(B):
                for m in range(2):
                    t = b * 2 + m
                    nc.vector.tensor_copy(st[:, t:t + 1], gsb[b][m][:, 256:257])
                    dtmp = small.tile([128, 128], F32, tag="dtmp", name=f"dtmp{t}")
                    nc.vector.tensor_mul(
                        dtmp[:], gsb[b][m][:, m * 128:(m + 1) * 128], ident[:]
                    )
                    with nc.allow_low_precision(reason="f32r output is bit-identical f32"):
                        nc.vector.reduce_sum(
                            out=st[:, 4 + t:5 + t], in_=dtmp[:], axis=mybir.AxisListType.X
                        )

            psum_g = p_misc.tile([4, 8], F32, tag="m", name="psum_g")
            nc.tensor.matmul(psum_g[:], konst[:, G4:G4 + 4], st[:], start=True, stop=True)
            gsb4 = small.tile([4, 8], F32, tag="gsb4", name="gsb4")
            nc.vector.tensor_copy(gsb4[:], psum_g[:])
            var44 = small.tile([4, 4], F32, tag="var44", name="var44")
            nc.vector.scalar_tensor_tensor(
                out=var44[:], in0=gsb4[:, 0:4], scalar=0.0, in1=gsb4[:, 0:4],
                op0=ALU.add, op1=ALU.mult,
            )  # mean^2
            nc.vector.tensor_sub(var44[:], gsb4[:, 4:8], var44[:])
            rstd44 = small.tile([4, 4], F32, tag="rstd44", name="rstd44")
            nc.scalar.activation(
                out=rstd44[:], in_=var44[:], func=ACT.Sqrt, bias=eps41[:], scale=1.0
            )
            with nc.allow_low_precision(reason="f32r output is bit-identical f32"):
                nc.vector.reciprocal(out=rstd44[:], in_=rstd44[:])

            a_sb = [[None] * 2 for _ in range(B)]
            bb_sb = [[None] * 2 for _ in range(B)]
            wts_sb = [[None] * 2 for _ in range(B)]
            e4f = small.tile([4, 128], F32, tag="e4f", name="e4f")
            nc.vector.tensor_copy(e4f[:], konst[0:4, E4:E4 + 128])
            e4 = e4f[:]
            for b in range(B):
                for cb in range(2):
                    t = b * 2 + cb
                    pmean = p_misc.tile([128, 1], F32, tag="m", name="pmean")
                    nc.tensor.matmul(
                        pmean[:], e4, gsb4[:, t:t + 1], start=True, stop=True
                    )
                    prstd = p_misc.tile([128, 1], F32, tag="m", name="prstd")
                    nc.tensor.matmul(
                        prstd[:], e4, rstd44[:, t:t + 1], start=True, stop=True
                    )
                    a = small.tile([128, 1], F32, tag=f"a{t}", name=f"a{t}")
                    nc.vector.tensor_mul(a[:], prstd[:], konst[:, GNW + cb:GNW + cb + 1])
                    na = small.tile([128, 1], F32, tag=f"na{t}", name=f"na{t}")
                    nc.scalar.mul(out=na[:], in_=a[:], mul=-1.0)
                    bbv = small.tile([128, 1], F32R, tag=f"bb{t}", name=f"bb{t}")
                    nc.vector.scalar_tensor_tensor(
                        out=bbv[:], in0=pmean[:], scalar=na[:],
                        in1=konst[:, GNB + cb:GNB + cb + 1],
                        op0=ALU.mult, op1=ALU.add,
                    )  # gnb - mean*a
                    w = wpool.tile([128, 512], F32R, tag=f"wts{t}", name=f"wts{t}")
                    nc.vector.tensor_scalar_mul(
                        out=w[:], in0=konst[:, WTQK0 + cb * 512:WTQK0 + (cb + 1) * 512],
                        scalar1=a[:],
                    )
                    a_sb[b][cb], bb_sb[b][cb], wts_sb[b][cb] = a, bbv, w

            # rank-1 stacks for the bias/shift logit corrections
            lq_sb, rk_sb = [], []
            for b in range(B):
                prb = p_misc.tile([1, 512], F32, tag="m", name="prb")
                nc.tensor.matmul(
                    prb[:], bb_sb[b][0][:], konst[:, WTQK0:WTQK0 + 512],
                    start=True, stop=False,
                )
                nc.tensor.matmul(
                    prb[:], bb_sb[b][1][:], konst[:, WTQK1:WTQK1 + 512],
                    start=False, stop=False,
                )
                nc.tensor.matmul(
                    prb[:], konst[0:1, ONE:ONE + 1], konst[0:1, BQK:BQK + 512],
                    start=False, stop=True,
                )
                rb = small.tile([1, 512], F32, tag=f"rb{b}", name=f"rb{b}")
                nc.vector.tensor_copy(rb[:], prb[:])
                psg = p_misc.tile([1, 512], F32, tag="m", name="psg")
                nc.tensor.matmul(
                    psg[:], st[:, b * 2:b * 2 + 1], wts_sb[b][0][:],
                    start=True, stop=False,
                )
                nc.tensor.matmul(
                    psg[:], st[:, b * 2 + 1:b * 2 + 2], wts_sb[b][1][:],
                    start=False, stop=True,
                )
                sg = small.tile([1, 512], F32, tag=f"sg{b}", name=f"sg{b}")
                nc.vector.tensor_copy(sg[:], psg[:])
                rbn = small.tile([1, 512], F32, tag=f"rbn{b}", name=f"rbn{b}")
                nc.scalar.mul(out=rbn[:], in_=rb[:], mul=float(N))
                lq = small.tile([3, 256], F32, tag=f"lq{b}", name=f"lq{b}")
                nc.sync.dma_start(out=lq[0:1, :], in_=rb[0:1, 0:256])
                nc.sync.dma_start(out=lq[1:2, :], in_=sg[0:1, 0:256])
                nc.sync.dma_start(out=lq[2:3, :], in_=rbn[0:1, 0:256])
                rk = small.tile([3, 256], F32, tag=f"rk{b}", name=f"rk{b}")
                nc.sync.dma_start(out=rk[0:1, :], in_=sg[0:1, 256:512])
                nc.sync.dma_start(out=rk[1:2, :], in_=rb[0:1, 256:512])
                nc.sync.dma_start(out=rk[2:3, :], in_=rb[0:1, 256:512])
                lq_sb.append(lq)
                rk_sb.append(rk)

            # ---------- phase B2: logits L_h = Wq_a G Wk_a^T + corrections ----------
            # Heads of a pair stack along PSUM COLUMNS (a matmul with a
            # 128-row stationary tile must write at partition 0).
            att_sm = [[None] * 4 for _ in range(B)]  # per head [64,64]
            for b in range(B):
                for hp in range(2):
                    l_ps = p_att.tile([64, 128], F32, tag="L", name=f"L{b}{hp}")
                    for hh in range(2):
                        h = 2 * hp + hh
                        hq = slice(h * 64, (h + 1) * 64)
                        hk = slice(256 + h * 64, 256 + (h + 1) * 64)
                        u_ps = p_misc.tile([64, 256], F32, tag="u", name=f"u{b}{h}")
                        nc.tensor.matmul(
                            u_ps[:], wts_sb[b][0][:, hq], gsb[b][0][:, 0:256],
                            start=True, stop=False,
                        )
                        nc.tensor.matmul(
                            u_ps[:], wts_sb[b][1][:, hq], gsb[b][1][:, 0:256],
                            start=False, stop=True,
                        )
                        usb = small.tile([64, 256], F32, tag="usb", name=f"usb{b}{h}")
                        nc.vector.tensor_copy(usb[:], u_ps[:])
                        utm = []
                        for m in range(2):
                            tp_ps = p_misc.tile([128, 64], F32, tag="u", name=f"tp{b}{h}{m}")
                            nc.tensor.transpose(
                                tp_ps[:], usb[:, m * 128:(m + 1) * 128], ident[0:64, 0:64]
                            )
                            ut = small.tile([128, 64], F32R, tag=f"ut{m}", name=f"ut{b}{h}{m}")
                            nc.scalar.copy(out=ut[:], in_=tp_ps[:])
                            utm.append(ut)
                        csl = slice(hh * 64, (hh + 1) * 64)
                        nc.tensor.matmul(
                            l_ps[:, csl], utm[0][:], wts_sb[b][0][:, hk],
                            start=True, stop=False, skip_group_check=True,
                        )
                        nc.tensor.matmul(
                            l_ps[:, csl], utm[1][:], wts_sb[b][1][:, hk],
                            start=False, stop=False, skip_group_check=True,
                        )
                        nc.tensor.matmul(
                            l_ps[:, csl], lq_sb[b][:, h * 64:(h + 1) * 64],
                            rk_sb[b][:, h * 64:(h + 1) * 64],
                            start=False, stop=True, skip_group_check=True,
                        )
                    for hh in range(2):
                        h = 2 * hp + hh
                        csl = slice(hh * 64, (hh + 1) * 64)
                        # softmax over each head's own 64 columns (scale SM_SCALE)
                        atc = small.tile([64, 64], F32, tag="atc", name=f"atc{b}{h}")
                        nc.vector.tensor_copy(atc[:], l_ps[:, csl])
                        negm = small.tile([64, 1], F32, tag="negm", name=f"negm{b}{h}")
                        nc.vector.reduce_max(
                            out=negm[:], in_=atc[:], axis=mybir.AxisListType.X, negate=True
                        )
                        nc.scalar.mul(out=negm[:], in_=negm[:], mul=SM_SCALE)
                        esb = small.tile([64, 64], F32, tag="esb", name=f"esb{b}{h}")
                        nc.scalar.activation(
                            out=esb[:], in_=atc[:], func=ACT.Exp,
                            bias=negm[:], scale=SM_SCALE,
                        )
                        ssum = small.tile([64, 1], F32, tag="ssum", name=f"ssum{b}{h}")
                        nc.vector.reduce_sum(out=ssum[:], in_=esb[:], axis=mybir.AxisListType.X)
                        nc.vector.reciprocal(out=ssum[:], in_=ssum[:])
                        sm = small.tile([64, 64], F32R, tag=f"sm{b}{h}", name=f"sm{b}{h}")
                        nc.vector.tensor_scalar_mul(out=sm[:], in0=esb[:], scalar1=ssum[:])
                        att_sm[b][h] = sm

            # ---------- phase B3: fused per-batch weights ----------
            gbt_sb = [[None] * 2 for _ in range(B)]
            mbt_sb = [[None] * 2 for _ in range(B)]
            beta_sb = [[None] * 2 for _ in range(B)]
            for b in range(B):
                ablk = []
                for k in range(2):
                    ab = wpool.tile([128, 256], F32R, tag=f"ablk{b}{k}", name=f"ablk{b}{k}")
                    nc.vector.tensor_copy(ab[:], konst[:, ZEROS:ZEROS + 256])
                    h0, h1 = 2 * k, 2 * k + 1
                    nc.vector.tensor_copy(ab[0:64, h0 * 64:(h0 + 1) * 64], att_sm[b][h0][:])
                    # odd head sits at partitions 0:64; DMA shifts it to 64:128
                    nc.sync.dma_start(
                        out=ab[64:128, h1 * 64:(h1 + 1) * 64], in_=att_sm[b][h1][:]
                    )
                    ablk.append(ab)
                for m in range(2):
                    pm = p_misc.tile([128, 256], F32, tag="u", name=f"pm{b}{m}")
                    msl = slice(m * 128, (m + 1) * 128)
                    nc.tensor.matmul(
                        pm[:], ablk[0][:, msl], konst[:, PT0:PT0 + 256],
                        start=True, stop=False,
                    )
                    nc.tensor.matmul(
                        pm[:], ablk[1][:, msl], konst[:, PT1:PT1 + 256],
                        start=False, stop=True,
                    )
                    mbt = wpool.tile([128, 256], F32R, tag=f"mbt{b}{m}", name=f"mbt{b}{m}")
                    nc.vector.tensor_copy(mbt[:], pm[:])
                    mbt_sb[b][m] = mbt
                for g in range(2):
                    pg2 = p_misc.tile([128, 256], F32, tag="u", name=f"pg2{b}{g}")
                    gsl = slice(g * 128, (g + 1) * 128)
                    nc.tensor.matmul(
                        pg2[:], konst[:, WV0 + gsl.start:WV0 + gsl.stop], mbt_sb[b][0][:],
                        start=True, stop=False,
                    )
                    nc.tensor.matmul(
                        pg2[:], konst[:, WV1 + gsl.start:WV1 + gsl.stop],
                        mbt_sb[b][1][:],
                        start=False, stop=True,
                    )
                    gbt = wpool.tile([128, 256], F32R, tag=f"gbt{b}{g}", name=f"gbt{b}{g}")
                    nc.vector.tensor_copy(gbt[:], pg2[:])
                    gbt_sb[b][g] = gbt
                pbeta = p_misc.tile([1, C], F32, tag="m", name=f"pbeta{b}")
                nc.tensor.matmul(pbeta[:], bb_sb[b][0][:], gbt_sb[b][0][:], start=True, stop=False)
                nc.tensor.matmul(pbeta[:], bb_sb[b][1][:], gbt_sb[b][1][:], start=False, stop=False)
                nc.tensor.matmul(pbeta[:], konst[:, BVC:BVC + 1], mbt_sb[b][0][:], start=False, stop=False)
                nc.tensor.matmul(pbeta[:], konst[:, BVC + 1:BVC + 2], mbt_sb[b][1][:], start=False, stop=True)
                brow = small.tile([1, C], F32, tag=f"brow{b}", name=f"brow{b}")
                nc.vector.tensor_add(brow[:], pbeta[:], konst[0:1, PB:PB + 256])
                for mo in range(2):
                    bet = small.tile([128, 1], F32, tag=f"beta{b}{mo}", name=f"beta{b}{mo}")
                    nc.sync.dma_start(out=bet[:], in_=brow[0:1, mo * 128:(mo + 1) * 128])
                    beta_sb[b][mo] = bet
                # fold the GroupNorm scale into G_b (after the bias matmuls read it)
                for g in range(2):
                    nc.vector.tensor_scalar_mul(
                        out=gbt_sb[b][g][:], in0=gbt_sb[b][g][:], scalar1=a_sb[b][g][:]
                    )

            # ---------- phase C: out = G_b' x + beta + x (streamed) ----------
            for b in range(B):
                for j in range(N // CT):
                    jsl = slice(j * CT, (j + 1) * CT)
                    x0 = xc0.tile([128, CT], F32R, tag="x0", name=f"x0_{b}{j}")
                    nc.sync.dma_start(out=x0[:], in_=xs_d[b * 2][:, jsl])
                    x1 = xc1.tile([128, CT], F32R, tag="x1", name=f"x1_{b}{j}")
                    nc.sync.dma_start(out=x1[:], in_=xs_d[b * 2 + 1][:, jsl])
                    xin = [x0, x1]
                    for mo in range(2):
                        msl = slice(mo * 128, (mo + 1) * 128)
                        osb = op.tile([128, CT], F32, tag=f"o{mo}", name=f"o{b}{j}{mo}")
                        for sub in range(CT // 512):
                            ssl = slice(sub * 512, (sub + 1) * 512)
                            po = p_big.tile([128, 512], F32, tag="w", name=f"po{b}{j}{mo}{sub}")
                            nc.tensor.matmul(
                                po[:], gbt_sb[b][0][:, msl], x0[:, ssl],
                                start=True, stop=False,
                            )
                            nc.tensor.matmul(
                                po[:], gbt_sb[b][1][:, msl], x1[:, ssl],
                                start=False, stop=True,
                            )
                            nc.vector.scalar_tensor_tensor(
                                out=osb[:, ssl], in0=po[:], scalar=beta_sb[b][mo][:],
                                in1=xin[mo][:, ssl], op0=ALU.add, op1=ALU.add,
                            )
                        nc.sync.dma_start(out=out_d[b * 2 + mo][:, jsl], in_=osb[:])

    _split_excess_waits(nc)
    return nc


_NC_CACHE = None


def _get_nc():
    global _NC_CACHE
    if _NC_CACHE is None:
        _NC_CACHE = build_nc()
    return _NC_CACHE


def _prep_inputs(x, gn_w, gn_b, qkv_w, qkv_b, proj_w, proj_b):
    x2 = np.ascontiguousarray(np.asarray(x, np.float32)).reshape(B, C, N)
    qkv_w = np.asarray(qkv_w, np.float32)
    qkv_b = np.asarray(qkv_b, np.float32)
    proj_w = np.asarray(proj_w, np.float32)
    proj_b = np.asarray(proj_b, np.float32)
    gn_w = np.asarray(gn_w, np.float32)
    gn_b = np.asarray(gn_b, np.float32)

    xs = np.ascontiguousarray(x2.reshape(B * 2, 128, N))

    # transposed copy with a fused ones-column: xt[b, p, j*257+q] = x[b, q, j*128+p]
    xtr = np.zeros((B, N, 258), np.float32)
    xtr[:, :, 0:256] = np.transpose(x2, (0, 2, 1))
    xtr[:, :, 256] = 1.0
    xt = np.ascontiguousarray(
        xtr.reshape(B, NT, 128, 258).transpose(0, 2, 1, 3).reshape(B, 128, NT * 258)
    )

    konst = np.zeros((128, KC), np.float32)
    wtqk = qkv_w[0:512].T                       # [256, 512]
    konst[:, WTQK0:WTQK0 + 512] = wtqk[0:128]
    konst[:, WTQK1:WTQK1 + 512] = wtqk[128:256]
    konst[:, WV0:WV0 + 256] = qkv_w[512:640]
    konst[:, WV1:WV1 + 256] = qkv_w[640:768]
    pt = proj_w.T
    konst[:, PT0:PT0 + 256] = pt[0:128]
    konst[:, PT1:PT1 + 256] = pt[128:256]
    konst[:, GNW] = gn_w[0:128]
    konst[:, GNW + 1] = gn_w[128:256]
    konst[:, GNB] = gn_b[0:128]
    konst[:, GNB + 1] = gn_b[128:256]
    konst[:, BVC] = qkv_b[512:640]
    konst[:, BVC + 1] = qkv_b[640:768]
    for p in range(128):
        konst[p, G4 + p // 32] = 1.0 / (32.0 * N)
        konst[p // 32, E4 + p] = 1.0
    konst[0, BQK:BQK + 512] = qkv_b[0:512]
    konst[0, PB:PB + 256] = proj_b
    konst[0, ONE] = 1.0

    return [{"xs": xs, "xt": xt, "konst": konst}]


def kernel(x, gn_w, gn_b, qkv_w, qkv_b, proj_w, proj_b):
    nc = _get_nc()
    in_maps = _prep_inputs(x, gn_w, gn_b, qkv_w, qkv_b, proj_w, proj_b)
    res = run_bass_kernel_spmd(nc, in_maps, [0], trace=False)
    out = res.results[0]["out"].reshape(B, C, N)
    return out.reshape(B, C, 32, 32, 32).astype(np.float32)


# revision 15
# speedup vs baseline: 1.5286x; 1.3500x over previous
"""Trainium2 Bass kernel for nn_Attention3D (GroupNorm + channel-attention + proj + residual).

Single-core design. Measurement showed the per-call wall time through the
axon/PJRT relay is dominated by a fixed ~68ms dispatch floor that grows with
the number of devices in the mesh (and with per-device buffer count), while
on-device compute for this problem is <1ms. So: run everything on ONE core
with zero collectives and only three input buffers.

Algorithm (validated against the reference in numpy, rel err ~2e-5):
  - Per batch, the channel Gram matrix G = x x^T [256,256] and row-sums
    s = x 1 are accumulated from a host-pretransposed copy of x (with a
    ones-column fused so one matmul pair yields both G and s).
  - GroupNorm stats come from s (means) and diag(G) (second moments), so the
    normalization is never materialized: its affine folds into the q/k
    weights (per-batch row scaling a) and rank-1 logit corrections.
  - Channel-attention logits = Wq_a G Wk_a^T + rank-1 corrections (from bias
    and GroupNorm shift terms), then per-head softmax on the [64,64] blocks.
  - softmax @ v + proj collapse into a per-batch weight G_b = P blockdiag(A)
    Wv (x scale fold), applied to raw x in one streamed pass with the
    residual and all bias terms fused.
"""
import sys

sys.path.insert(0, "/opt/trn_rl_repo")

import numpy as np
import concourse.bass as bass
import concourse.tile as tile
from concourse import mybir
from concourse.bass_utils import run_bass_kernel_spmd
from concourse.masks import make_identity

F32 = mybir.dt.float32
F32R = mybir.dt.float32r
ALU = mybir.AluOpType
ACT = mybir.ActivationFunctionType

B, C = 2, 256
N = 32 * 32 * 32
H, HD = 4, 64
G = 8
EPS = 1e-5
SM_SCALE = float(HD) ** -0.5
NT = N // 128          # 256 transposed n-tiles per batch
TB = 4                 # n-tiles per DMA in the Gram pass
CT = 1024              # column tile in the output pass

# konst column layout
WTQK0, WTQK1 = 0, 512            # [128,512] each: (qkv_w[0:512].T) chunks
WV0, WV1 = 1024, 1280            # [128,256] each: qkv_w[512:768] row chunks
PT0, PT1 = 1536, 1792            # [128,256] each: proj_w.T row chunks
ZEROS = 2048                     # [128,256] zeros
GNW, GNB, BVC = 2304, 2306, 2308  # [128,1] x2 each
G4 = 2310                        # [128,4] group reducer (1/(32N))
E4 = 2314                        # rows 0:4, [4,128] group broadcaster
BQK = 2442                       # rows 0:1, [1,512] qk bias
PB = 2954                        # rows 0:1, [1,256] proj bias
ONE = 3210                       # rows 0:1, [1,1] = 1.0
ONESC = 3212                     # [128,1] ones column
ONESR = 3213                     # rows 0:1, [1,128] ones row
KC = 3342


def _split_excess_waits(nc, max_waits=1):
    """This container's walrus rejects >1 sem wait per instruction; move the
    overflow onto same-engine NoOps inserted immediately before."""
    ctr = 0
    for bb in nc.cur_f.blocks:
        insts = bb.instructions
        i = 0
        while i < len(insts):
            ins = insts[i]
            si = ins.sync_info
            if si is not None and len(si.on_wait) > max_waits:
                waits = list(si.on_wait)
                si.on_wait = waits[:max_waits]
                overflow = waits[max_waits:]
                pos = i
                for j in range(0, len(overflow), max_waits):
                    ctr += 1
                    nop = mybir.InstNoOp(name=f"I-ws-{ctr}", ins=[], outs=[])
                    nop.engine = ins.engine
                    nop.sync_info = mybir.SyncInfo(
                        on_wait=overflow[j : j + max_waits], on_update=[]
                    )
                    insts.insert(pos, nop)
                    pos += 1
                    i += 1
            i += 1


def build_nc():
    nc = bass.Bass(num_devices=1)

    xs_d = nc.declare_dram_parameter("xs", [2 * B, 128, N], F32R, isOutput=False)
    xt_d = nc.declare_dram_parameter("xt", [B, 128, NT * 258], F32R, isOutput=False)
    konst_d = nc.declare_dram_parameter("konst", [128, KC], F32R, isOutput=False)
    out_d = nc.declare_dram_parameter("out", [2 * B, 128, N], F32, isOutput=True)

    with tile.TileContext(nc) as tc:
        with (
            tc.tile_pool(name="wpool", bufs=1) as wpool,    # konst, wts, G, fused mats
            tc.tile_pool(name="small", bufs=1) as small,    # stats / vectors
            tc.tile_pool(name="xtp", bufs=3) as xtp,        # Gram-pass staging
            tc.tile_pool(name="xc0", bufs=2) as xc0,        # out-pass x chunk0
            tc.tile_pool(name="xc1", bufs=2) as xc1,        # out-pass x chunk1
            tc.tile_pool(name="op", bufs=2) as op,          # out-pass staging
            tc.tile_pool(name="p_big", bufs=2, space="PSUM") as p_big,
            tc.tile_pool(name="p_att", bufs=1, space="PSUM") as p_att,
            tc.tile_pool(name="p_misc", bufs=2, space="PSUM") as p_misc,
        ):
            # ---------- loads / constants ----------
            konst = wpool.tile([128, KC], F32R, tag="konst", name="konst")
            nc.sync.dma_start(out=konst[:], in_=konst_d[:])
            ident = small.tile([128, 128], F32, tag="ident", name="ident")
            make_identity(nc, ident[:])
            eps41 = small.tile([4, 1], F32, tag="eps", name="eps")
            nc.gpsimd.memset(eps41[:], EPS)
            scr41 = small.tile([4, 1], F32, tag="scr", name="scr")
            # preload activation tables while DMAs run
            nc.scalar.activation(out=scr41[:], in_=eps41[:], func=ACT.Sqrt)
            nc.scalar.activation(out=scr41[:], in_=eps41[:], func=ACT.Exp)

            # ---------- phase A: per-batch Gram G = x x^T and row-sums ----------
            gsb = [[None] * 2 for _ in range(B)]  # [b][half] -> [128,258] SBUF
            for b in range(B):
                g_ps = [
                    p_big.tile([128, 258], F32, tag="w", name=f"g{b}{m}")
                    for m in range(2)
                ]
                n_batches = NT // TB
                for jb in range(n_batches):
                    xt = xtp.tile([128, TB * 258], F32R, tag="xt", name=f"xt{b}{jb}")
                    nc.sync.dma_start(
                        out=xt[:], in_=xt_d[b][:, jb * TB * 258:(jb + 1) * TB * 258]
                    )
                    for k in range(TB):
                        first = jb == 0 and k == 0
                        last = jb == n_batches - 1 and k == TB - 1
                        base = k * 258
                        for m in range(2):
                            nc.tensor.matmul(
                                g_ps[m][:],
                                xt[:, base + m * 128: base + (m + 1) * 128],
                                xt[:, base: base + 258],
                                start=first, stop=last,
                            )
                for m in range(2):
                    gt = wpool.tile([128, 258], F32, tag=f"G{b}{m}", name=f"G{b}{m}")
                    if m == 0:
                        nc.vector.tensor_copy(gt[:], g_ps[m][:])
                    else:
                        nc.scalar.copy(out=gt[:], in_=g_ps[m][:])
                    gsb[b][m] = gt

            # ---------- phase B1: GroupNorm stats from s and diag(G) ----------
            st = small.tile([128, 8], F32R, tag="st", name="st")
            for b in range(B):
                for m in range(2):
                    t = b * 2 + m
                    nc.vector.tensor_copy(st[:, t:t + 1], gsb[b][m][:, 256:257])
                    dtmp = small.tile([128, 128], F32, tag="dtmp", name=f"dtmp{t}")
                    nc.vector.tensor_mul(
                        dtmp[:], gsb[b][m][:, m * 128:(m + 1) * 128], ident[:]
                    )
                    with nc.allow_low_precision(reason="f32r output is bit-identical f32"):
                        nc.vector.reduce_sum(
                            out=st[:, 4 + t:5 + t], in_=dtmp[:], axis=mybir.AxisListType.X
                        )

            # mean of diag(G) per batch: gbar_b; center G's diagonal by it
            # (exact: Wqa G Wka^T = Wqa G' Wka^T + gbar * Wqa Wka^T, G' = G - gbar I)
            onesr_f = small.tile([1, 128], F32, tag="onesr_f", name="onesr_f")
            nc.vector.tensor_copy(onesr_f[:], konst[0:1, ONESR:ONESR + 128])
            gd_ps = p_misc.tile([1, 4], F32, tag="m", name="gd_ps")
            nc.tensor.matmul(
                gd_ps[:], konst[:, ONESC:ONESC + 1], st[:, 4:8], start=True, stop=True
            )
            gdsum = small.tile([1, 4], F32, tag="gdsum", name="gdsum")
            nc.vector.tensor_copy(gdsum[:], gd_ps[:])
            gb64_sb, ng128_sb = [], []
            for b in range(B):
                gsum = small.tile([1, 1], F32, tag=f"gsum{b}", name=f"gsum{b}")
                nc.vector.tensor_add(
                    gsum[:], gdsum[0:1, 2 * b:2 * b + 1], gdsum[0:1, 2 * b + 1:2 * b + 2]
                )
                gbar = small.tile([1, 1], F32, tag=f"gbar{b}", name=f"gbar{b}")
                nc.scalar.mul(out=gbar[:], in_=gsum[:], mul=1.0 / 256.0)
                negg = small.tile([1, 1], F32, tag=f"negg{b}", name=f"negg{b}")
                nc.scalar.mul(out=negg[:], in_=gsum[:], mul=-1.0 / 256.0)
                bc_ps = p_misc.tile([64, 1], F32, tag="m", name=f"bc64{b}")
                nc.tensor.matmul(bc_ps[:], onesr_f[0:1, 0:64], gbar[:], start=True, stop=True)
                gb64 = small.tile([64, 1], F32, tag=f"gb64{b}", name=f"gb64{b}")
                nc.vector.tensor_copy(gb64[:], bc_ps[:])
                gb64_sb.append(gb64)
                bn_ps = p_misc.tile([128, 1], F32, tag="m", name=f"bc128{b}")
                nc.tensor.matmul(bn_ps[:], onesr_f[:], negg[:], start=True, stop=True)
                ng128 = small.tile([128, 1], F32, tag=f"ng128{b}", name=f"ng128{b}")
                nc.vector.tensor_copy(ng128[:], bn_ps[:])
                ng128_sb.append(ng128)
            for b in range(B):
                for m in range(2):
                    msl = slice(m * 128, (m + 1) * 128)
                    nc.vector.scalar_tensor_tensor(
                        out=gsb[b][m][:, msl], in0=ident[:], scalar=ng128_sb[b][:],
                        in1=gsb[b][m][:, msl], op0=ALU.mult, op1=ALU.add,
                    )

            psum_g = p_misc.tile([4, 8], F32, tag="m", name="psum_g")
            nc.tensor.matmul(psum_g[:], konst[:, G4:G4 + 4], st[:], start=True, stop=True)
            gsb4 = small.tile([4, 8], F32, tag="gsb4", name="gsb4")
            nc.vector.tensor_copy(gsb4[:], psum_g[:])
            var44 = small.tile([4, 4], F32, tag="var44", name="var44")
            nc.vector.scalar_tensor_tensor(
                out=var44[:], in0=gsb4[:, 0:4], scalar=0.0, in1=gsb4[:, 0:4],
                op0=ALU.add, op1=ALU.mult,
            )  # mean^2
            nc.vector.tensor_sub(var44[:], gsb4[:, 4:8], var44[:])
            rstd44 = small.tile([4, 4], F32, tag="rstd44", name="rstd44")
            nc.scalar.activation(
                out=rstd44[:], in_=var44[:], func=ACT.Sqrt, bias=eps41[:], scale=1.0
            )
            with nc.allow_low_precision(reason="f32r output is bit-identical f32"):
                nc.vector.reciprocal(out=rstd44[:], in_=rstd44[:])

            a_sb = [[None] * 2 for _ in range(B)]
            bb_sb = [[None] * 2 for _ in range(B)]
            wts_sb = [[None] * 2 for _ in range(B)]
            wtsf_sb = [[None] * 2 for _ in range(B)]
            e4f = small.tile([4, 128], F32, tag="e4f", name="e4f")
            nc.vector.tensor_copy(e4f[:], konst[0:4, E4:E4 + 128])
            e4 = e4f[:]
            for b in range(B):
                for cb in range(2):
                    t = b * 2 + cb
                    pmean = p_misc.tile([128, 1], F32, tag="m", name="pmean")
                    nc.tensor.matmul(
                        pmean[:], e4, gsb4[:, t:t + 1], start=True, stop=True
                    )
                    prstd = p_misc.tile([128, 1], F32, tag="m", name="prstd")
                    nc.tensor.matmul(
                        prstd[:], e4, rstd44[:, t:t + 1], start=True, stop=True
                    )
                    a = small.tile([128, 1], F32, tag=f"a{t}", name=f"a{t}")
                    nc.vector.tensor_mul(a[:], prstd[:], konst[:, GNW + cb:GNW + cb + 1])
                    na = small.tile([128, 1], F32, tag=f"na{t}", name=f"na{t}")
                    nc.scalar.mul(out=na[:], in_=a[:], mul=-1.0)
                    bbv = small.tile([128, 1], F32R, tag=f"bb{t}", name=f"bb{t}")
                    nc.vector.scalar_tensor_tensor(
                        out=bbv[:], in0=pmean[:], scalar=na[:],
                        in1=konst[:, GNB + cb:GNB + cb + 1],
                        op0=ALU.mult, op1=ALU.add,
                    )  # gnb - mean*a
                    w = wpool.tile([128, 512], F32R, tag=f"wts{t}", name=f"wts{t}")
                    nc.vector.tensor_scalar_mul(
                        out=w[:], in0=konst[:, WTQK0 + cb * 512:WTQK0 + (cb + 1) * 512],
                        scalar1=a[:],
                    )
                    wf = wpool.tile([128, 512], F32, tag=f"wtsf{t}", name=f"wtsf{t}")
                    nc.scalar.copy(out=wf[:], in_=w[:])
                    wtsf_sb[b][cb] = wf
                    a_sb[b][cb], bb_sb[b][cb], wts_sb[b][cb] = a, bbv, w

            # rank-1 stacks for the bias/shift logit corrections
            lq_sb, rk_sb = [], []
            for b in range(B):
                prb = p_misc.tile([1, 512], F32, tag="m", name="prb")
                nc.tensor.matmul(
                    prb[:], bb_sb[b][0][:], konst[:, WTQK0:WTQK0 + 512],
                    start=True, stop=False,
                )
                nc.tensor.matmul(
                    prb[:], bb_sb[b][1][:], konst[:, WTQK1:WTQK1 + 512],
                    start=False, stop=False,
                )
                nc.tensor.matmul(
                    prb[:], konst[0:1, ONE:ONE + 1], konst[0:1, BQK:BQK + 512],
                    start=False, stop=True,
                )
                rb = small.tile([1, 512], F32, tag=f"rb{b}", name=f"rb{b}")
                nc.vector.tensor_copy(rb[:], prb[:])
                psg = p_misc.tile([1, 512], F32, tag="m", name="psg")
                nc.tensor.matmul(
                    psg[:], st[:, b * 2:b * 2 + 1], wts_sb[b][0][:],
                    start=True, stop=False,
                )
                nc.tensor.matmul(
                    psg[:], st[:, b * 2 + 1:b * 2 + 2], wts_sb[b][1][:],
                    start=False, stop=True,
                )
                sg = small.tile([1, 512], F32, tag=f"sg{b}", name=f"sg{b}")
                nc.vector.tensor_copy(sg[:], psg[:])
                rbn = small.tile([1, 512], F32, tag=f"rbn{b}", name=f"rbn{b}")
                nc.scalar.mul(out=rbn[:], in_=rb[:], mul=float(N))
                lq = small.tile([3, 256], F32, tag=f"lq{b}", name=f"lq{b}")
                nc.sync.dma_start(out=lq[0:1, :], in_=rb[0:1, 0:256])
                nc.sync.dma_start(out=lq[1:2, :], in_=sg[0:1, 0:256])
                nc.sync.dma_start(out=lq[2:3, :], in_=rbn[0:1, 0:256])
                rk = small.tile([3, 256], F32, tag=f"rk{b}", name=f"rk{b}")
                nc.sync.dma_start(out=rk[0:1, :], in_=sg[0:1, 256:512])
                nc.sync.dma_start(out=rk[1:2, :], in_=rb[0:1, 256:512])
                nc.sync.dma_start(out=rk[2:3, :], in_=rb[0:1, 256:512])
                lq_sb.append(lq)
                rk_sb.append(rk)

            # ---------- phase B2: logits L_h = Wq_a G Wk_a^T + corrections ----------
            # Heads of a pair stack along PSUM COLUMNS (a matmul with a
            # 128-row stationary tile must write at partition 0).
            att_sm = [[None] * 4 for _ in range(B)]  # per head [64,64]
            for b in range(B):
                for hp in range(2):
                    l_ps = p_att.tile([64, 128], F32, tag="L", name=f"L{b}{hp}")
                    pqk_sb = [None, None]
                    for hh in range(2):
                        h = 2 * hp + hh
                        hq = slice(h * 64, (h + 1) * 64)
                        hk = slice(256 + h * 64, 256 + (h + 1) * 64)
                        u_ps = p_misc.tile([64, 256], F32, tag="u", name=f"u{b}{h}")
                        nc.tensor.matmul(
                            u_ps[:], wtsf_sb[b][0][:, hq], gsb[b][0][:, 0:256],
                            start=True, stop=False,
                        )
                        nc.tensor.matmul(
                            u_ps[:], wtsf_sb[b][1][:, hq], gsb[b][1][:, 0:256],
                            start=False, stop=True,
                        )
                        usb = small.tile([64, 256], F32, tag="usb", name=f"usb{b}{h}")
                        nc.vector.tensor_copy(usb[:], u_ps[:])
                        utm = []
                        for m in range(2):
                            tp_ps = p_misc.tile([128, 64], F32, tag="u", name=f"tp{b}{h}{m}")
                            nc.tensor.transpose(
                                tp_ps[:], usb[:, m * 128:(m + 1) * 128], ident[0:64, 0:64]
                            )
                            ut = small.tile([128, 64], F32, tag=f"ut{m}", name=f"ut{b}{h}{m}")
                            nc.scalar.copy(out=ut[:], in_=tp_ps[:])
                            utm.append(ut)
                        csl = slice(hh * 64, (hh + 1) * 64)
                        nc.tensor.matmul(
                            l_ps[:, csl], utm[0][:], wtsf_sb[b][0][:, hk],
                            start=True, stop=False, skip_group_check=True,
                        )
                        nc.tensor.matmul(
                            l_ps[:, csl], utm[1][:], wtsf_sb[b][1][:, hk],
                            start=False, stop=False, skip_group_check=True,
                        )
                        nc.tensor.matmul(
                            l_ps[:, csl], lq_sb[b][:, h * 64:(h + 1) * 64],
                            rk_sb[b][:, h * 64:(h + 1) * 64],
                            start=False, stop=True, skip_group_check=True,
                        )
                        pq_ps = p_misc.tile([64, 64], F32, tag="u", name=f"pq{b}{h}")
                        nc.tensor.matmul(
                            pq_ps[:], wtsf_sb[b][0][:, hq], wtsf_sb[b][0][:, hk],
                            start=True, stop=False,
                        )
                        nc.tensor.matmul(
                            pq_ps[:], wtsf_sb[b][1][:, hq], wtsf_sb[b][1][:, hk],
                            start=False, stop=True,
                        )
                        pqk = small.tile([64, 64], F32, tag=f"pqk{hh}", name=f"pqk{b}{h}")
                        nc.vector.tensor_copy(pqk[:], pq_ps[:])
                        pqk_sb[hh] = pqk
                    for hh in range(2):
                        h = 2 * hp + hh
                        csl = slice(hh * 64, (hh + 1) * 64)
                        # softmax over each head's own 64 columns (scale SM_SCALE);
                        # atc = L' + gbar * (Wqa Wka^T) restores the centered diagonal
                        atc = small.tile([64, 64], F32, tag="atc", name=f"atc{b}{h}")
                        nc.vector.scalar_tensor_tensor(
                            out=atc[:], in0=pqk_sb[hh][:], scalar=gb64_sb[b][:],
                            in1=l_ps[:, csl], op0=ALU.mult, op1=ALU.add,
                        )
                        negm = small.tile([64, 1], F32, tag="negm", name=f"negm{b}{h}")
                        nc.vector.reduce_max(
                            out=negm[:], in_=atc[:], axis=mybir.AxisListType.X, negate=True
                        )
                        nc.scalar.mul(out=negm[:], in_=negm[:], mul=SM_SCALE)
                        esb = small.tile([64, 64], F32, tag="esb", name=f"esb{b}{h}")
                        nc.scalar.activation(
                            out=esb[:], in_=atc[:], func=ACT.Exp,
                            bias=negm[:], scale=SM_SCALE,
                        )
                        ssum = small.tile([64, 1], F32, tag="ssum", name=f"ssum{b}{h}")
                        nc.vector.reduce_sum(out=ssum[:], in_=esb[:], axis=mybir.AxisListType.X)
                        nc.vector.reciprocal(out=ssum[:], in_=ssum[:])
                        sm = small.tile([64, 64], F32R, tag=f"sm{b}{h}", name=f"sm{b}{h}")
                        nc.vector.tensor_scalar_mul(out=sm[:], in0=esb[:], scalar1=ssum[:])
                        att_sm[b][h] = sm

            # ---------- phase B3: fused per-batch weights ----------
            gbt_sb = [[None] * 2 for _ in range(B)]
            mbt_sb = [[None] * 2 for _ in range(B)]
            beta_sb = [[None] * 2 for _ in range(B)]
            for b in range(B):
                ablk = []
                for k in range(2):
                    ab = wpool.tile([128, 256], F32R, tag=f"ablk{b}{k}", name=f"ablk{b}{k}")
                    nc.vector.tensor_copy(ab[:], konst[:, ZEROS:ZEROS + 256])
                    h0, h1 = 2 * k, 2 * k + 1
                    nc.vector.tensor_copy(ab[0:64, h0 * 64:(h0 + 1) * 64], att_sm[b][h0][:])
                    # odd head sits at partitions 0:64; DMA shifts it to 64:128
                    nc.sync.dma_start(
                        out=ab[64:128, h1 * 64:(h1 + 1) * 64], in_=att_sm[b][h1][:]
                    )
                    ablk.append(ab)
                for m in range(2):
                    pm = p_misc.tile([128, 256], F32, tag="u", name=f"pm{b}{m}")
                    msl = slice(m * 128, (m + 1) * 128)
                    nc.tensor.matmul(
                        pm[:], ablk[0][:, msl], konst[:, PT0:PT0 + 256],
                        start=True, stop=False,
                    )
                    nc.tensor.matmul(
                        pm[:], ablk[1][:, msl], konst[:, PT1:PT1 + 256],
                        start=False, stop=True,
                    )
                    mbt = wpool.tile([128, 256], F32R, tag=f"mbt{b}{m}", name=f"mbt{b}{m}")
                    nc.vector.tensor_copy(mbt[:], pm[:])
                    mbt_sb[b][m] = mbt
                for g in range(2):
                    pg2 = p_misc.tile([128, 256], F32, tag="u", name=f"pg2{b}{g}")
                    gsl = slice(g * 128, (g + 1) * 128)
                    nc.tensor.matmul(
                        pg2[:], konst[:, WV0 + gsl.start:WV0 + gsl.stop], mbt_sb[b][0][:],
                        start=True, stop=False,
                    )
                    nc.tensor.matmul(
                        pg2[:], konst[:, WV1 + gsl.start:WV1 + gsl.stop],
                        mbt_sb[b][1][:],
                        start=False, stop=True,
                    )
                    gbt = wpool.tile([128, 256], F32R, tag=f"gbt{b}{g}", name=f"gbt{b}{g}")
                    nc.vector.tensor_copy(gbt[:], pg2[:])
                    gbt_sb[b][g] = gbt
                pbeta = p_misc.tile([1, C], F32, tag="m", name=f"pbeta{b}")
                nc.tensor.matmul(pbeta[:], bb_sb[b][0][:], gbt_sb[b][0][:], start=True, stop=False)
                nc.tensor.matmul(pbeta[:], bb_sb[b][1][:], gbt_sb[b][1][:], start=False, stop=False)
                nc.tensor.matmul(pbeta[:], konst[:, BVC:BVC + 1], mbt_sb[b][0][:], start=False, stop=False)
                nc.tensor.matmul(pbeta[:], konst[:, BVC + 1:BVC + 2], mbt_sb[b][1][:], start=False, stop=True)
                brow = small.tile([1, C], F32, tag=f"brow{b}", name=f"brow{b}")
                nc.vector.tensor_add(brow[:], pbeta[:], konst[0:1, PB:PB + 256])
                for mo in range(2):
                    bet = small.tile([128, 1], F32, tag=f"beta{b}{mo}", name=f"beta{b}{mo}")
                    nc.sync.dma_start(out=bet[:], in_=brow[0:1, mo * 128:(mo + 1) * 128])
                    beta_sb[b][mo] = bet
                # fold the GroupNorm scale into G_b (after the bias matmuls read it)
                for g in range(2):
                    nc.vector.tensor_scalar_mul(
                        out=gbt_sb[b][g][:], in0=gbt_sb[b][g][:], scalar1=a_sb[b][g][:]
                    )

            # ---------- phase C: out = G_b' x + beta + x (streamed) ----------
            for b in range(B):
                for j in range(N // CT):
                    jsl = slice(j * CT, (j + 1) * CT)
                    x0 = xc0.tile([128, CT], F32R, tag="x0", name=f"x0_{b}{j}")
                    nc.sync.dma_start(out=x0[:], in_=xs_d[b * 2][:, jsl])
                    x1 = xc1.tile([128, CT], F32R, tag="x1", name=f"x1_{b}{j}")
                    nc.sync.dma_start(out=x1[:], in_=xs_d[b * 2 + 1][:, jsl])
                    xin = [x0, x1]
                    for mo in range(2):
                        msl = slice(mo * 128, (mo + 1) * 128)
                        osb = op.tile([128, CT], F32, tag=f"o{mo}", name=f"o{b}{j}{mo}")
                        for sub in range(CT // 512):
                            ssl = slice(sub * 512, (sub + 1) * 512)
                            po = p_big.tile([128, 512], F32, tag="w", name=f"po{b}{j}{mo}{sub}")
                            nc.tensor.matmul(
                                po[:], gbt_sb[b][0][:, msl], x0[:, ssl],
                                start=True, stop=False,
                            )
                            nc.tensor.matmul(
                                po[:], gbt_sb[b][1][:, msl], x1[:, ssl],
                                start=False, stop=True,
                            )
                            nc.vector.scalar_tensor_tensor(
                                out=osb[:, ssl], in0=po[:], scalar=beta_sb[b][mo][:],
                                in1=xin[mo][:, ssl], op0=ALU.add, op1=ALU.add,
                            )
                        nc.sync.dma_start(out=out_d[b * 2 + mo][:, jsl], in_=osb[:])

    _split_excess_waits(nc)
    return nc


_NC_CACHE = None


def _get_nc():
    global _NC_CACHE
    if _NC_CACHE is None:
        _NC_CACHE = build_nc()
    return _NC_CACHE


def _prep_inputs(x, gn_w, gn_b, qkv_w, qkv_b, proj_w, proj_b):
    x2 = np.ascontiguousarray(np.asarray(x, np.float32)).reshape(B, C, N)
    qkv_w = np.asarray(qkv_w, np.float32)
    qkv_b = np.asarray(qkv_b, np.float32)
    proj_w = np.asarray(proj_w, np.float32)
    proj_b = np.asarray(proj_b, np.float32)
    gn_w = np.asarray(gn_w, np.float32)
    gn_b = np.asarray(gn_b, np.float32)

    xs = np.ascontiguousarray(x2.reshape(B * 2, 128, N))

    # transposed copy with a fused ones-column (padded to 258 for fp32r ISA
    # evenness): xt[b, p, j*258+q] = x[b, q, j*128+p] for q<256; col 256 = 1, 257 = 0
    xtr = np.zeros((B, N, 258), np.float32)
    xtr[:, :, 0:256] = np.transpose(x2, (0, 2, 1))
    xtr[:, :, 256] = 1.0
    xt = np.ascontiguousarray(
        xtr.reshape(B, NT, 128, 258).transpose(0, 2, 1, 3).reshape(B, 128, NT * 258)
    )

    konst = np.zeros((128, KC), np.float32)
    wtqk = qkv_w[0:512].T                       # [256, 512]
    konst[:, WTQK0:WTQK0 + 512] = wtqk[0:128]
    konst[:, WTQK1:WTQK1 + 512] = wtqk[128:256]
    konst[:, WV0:WV0 + 256] = qkv_w[512:640]
    konst[:, WV1:WV1 + 256] = qkv_w[640:768]
    pt = proj_w.T
    konst[:, PT0:PT0 + 256] = pt[0:128]
    konst[:, PT1:PT1 + 256] = pt[128:256]
    konst[:, GNW] = gn_w[0:128]
    konst[:, GNW + 1] = gn_w[128:256]
    konst[:, GNB] = gn_b[0:128]
    konst[:, GNB + 1] = gn_b[128:256]
    konst[:, BVC] = qkv_b[512:640]
    konst[:, BVC + 1] = qkv_b[640:768]
    for p in range(128):
        konst[p, G4 + p // 32] = 1.0 / (32.0 * N)
        konst[p // 32, E4 + p] = 1.0
    konst[0, BQK:BQK + 512] = qkv_b[0:512]
    konst[0, PB:PB + 256] = proj_b
    konst[0, ONE] = 1.0
    konst[:, ONESC] = 1.0
    konst[0, ONESR:ONESR + 128] = 1.0

    return [{"xs": xs, "xt": xt, "konst": konst}]


def kernel(x, gn_w, gn_b, qkv_w, qkv_b, proj_w, proj_b):
    nc = _get_nc()
    in_maps = _prep_inputs(x, gn_w, gn_b, qkv_w, qkv_b, proj_w, proj_b)
    res = run_bass_kernel_spmd(nc, in_maps, [0], trace=False)
    out = res.results[0]["out"].reshape(B, C, N)
    return out.reshape(B, C, 32, 32, 32).astype(np.float32)
